# revision 7
# baseline (speedup 1.0000x reference)
"""Trainium2 Bass kernel for nn_ExpertGroup (moe_routing).

Contract: kernel(**inputs) takes FULL unsharded numpy inputs and returns the
FULL [2, 2048, 1024] fp32 output. Internally shards B*S=4096 tokens across
8 NeuronCores (512 tokens/core; cores 0-3 own batch 0, cores 4-7 batch 1),
replicates the small weights, and exchanges the per-batch adapter tensors
(adapt_in / adapt_out, [S,128] each) with two intra-group AllGathers.

All matmuls run in bf16 with fp32 PSUM accumulation. Activations are laid out
feature-major ([feature, token]) so every matmul contracts over partitions.

Host-side algebraic folds (exact, input-dependent, valid for any inputs):
  - up/gate weights concatenated into one [D, 2H] lhsT
  - shared_out + out = hidden @ down_w.T + adapt @ (0.1*down_w@adapt_proj_w).T
                      + mixed @ (output_proj_w@expert_proj_w).T
    -> single PSUM accumulation over 16+1+1 K-chunks of 128
  - sum_e ew[n,e]*adapter_b[e,:]  (LN bias term) = expert_weights @ adapter_b
"""

import sys

sys.path.insert(0, "/opt/trn_rl_repo")

import ml_dtypes
import numpy as np

import concourse.bass as bass
import concourse.mybir as mybir
import concourse.tile as tile
from concourse import bacc
from concourse.bass_utils import run_bass_kernel_spmd
from concourse.masks import make_identity

BF16 = mybir.dt.bfloat16
F32 = mybir.dt.float32

B, S, D, E = 2, 2048, 1024, 8
H = 2 * D          # 2048
A = H // 16        # 128
N = B * S          # 4096
NCORES = 8
T = N // NCORES    # 512 tokens per core
GROUP = 4          # cores per batch
SC = T // 128      # 4 s-chunks per core
DC = D // 128      # 8 d-chunks (output features)
HC = H // 128      # 16 h-chunks
KD = D // 128      # 8 k-chunks over D
TC_FULL = S // 128  # 16 token-chunks per batch
EPS = 1e-5

_CACHE = {}


def _build():
    nc = bacc.Bacc(None, num_devices=NCORES)

    # ---- kernel I/O (per-core) ----
    xT_d = nc.dram_tensor("xT", [D, T], BF16, kind="ExternalInput")
    ug_d = nc.dram_tensor("ug_wT", [D, 2 * H], BF16, kind="ExternalInput")
    pre_d = nc.dram_tensor("pre_wT", [D, A], BF16, kind="ExternalInput")
    post_d = nc.dram_tensor("post_wT", [H, A], BF16, kind="ExternalInput")
    adw_d = nc.dram_tensor("adapter_wT", [A, E * A], BF16, kind="ExternalInput")
    wfin_d = nc.dram_tensor("wfin", [H + 2 * A, D], BF16, kind="ExternalInput")
    ew_d = nc.dram_tensor("ew", [T, E], F32, kind="ExternalInput")
    angb_d = nc.dram_tensor("angb", [2, A], F32, kind="ExternalInput")
    ag_d = nc.dram_tensor("ag_row", [1, A * E], BF16, kind="ExternalInput")
    bmix_d = nc.dram_tensor("bias_mix", [T, A], BF16, kind="ExternalInput")
    out_d = nc.dram_tensor("out", [D, T], F32, kind="ExternalOutput")

    # ---- collective bounce buffers (internal DRAM) ----
    ag1_in = nc.dram_tensor("ag1_in", [T, A], BF16)
    ag1_out = nc.dram_tensor("ag1_out", [S, A], BF16)
    ag2_in = nc.dram_tensor("ag2_in", [A, T], BF16)
    ag2_out = nc.dram_tensor("ag2_out", [GROUP * A, T], BF16)
    RG = [[0, 1, 2, 3], [4, 5, 6, 7]]

    with tile.TileContext(nc) as tc:
        with (
            tc.tile_pool(name="consts", bufs=1) as consts,
            tc.tile_pool(name="wpool", bufs=1) as wpool,
            tc.tile_pool(name="acts", bufs=1) as acts,
            tc.tile_pool(name="work", bufs=4) as work,
            tc.tile_pool(name="work2", bufs=3) as work2,
            tc.tile_pool(name="workbig", bufs=2) as workbig,
            tc.tile_pool(name="evac", bufs=3) as evac,
            tc.tile_pool(name="ps_big", bufs=2, space="PSUM") as ps_big,
            tc.tile_pool(name="ps_acc", bufs=1, space="PSUM") as ps_acc,
            tc.tile_pool(name="ps_out", bufs=3, space="PSUM") as ps_out,
            tc.tile_pool(name="ps_sm", bufs=2, space="PSUM") as ps_sm,
        ):
            # ---------- priority loads: the pre-matmul chain needs these ----
            xT = wpool.tile([128, KD, T], BF16)
            pre_w = wpool.tile([128, KD, A], BF16)
            nc.sync.dma_start(out=xT, in_=xT_d[:].rearrange("(k p) t -> p k t", p=128))
            nc.sync.dma_start(
                out=pre_w, in_=pre_d[:].rearrange("(k p) a -> p k a", p=128)
            )

            # ---------- constants ----------
            ident = consts.tile([128, 128], BF16)
            make_identity(nc, ident)
            eps_t = consts.tile([128, 1], F32)
            nc.vector.memset(eps_t, EPS)
            gB = consts.tile([128, A], F32)   # an_g broadcast across partitions
            bB = consts.tile([128, A], F32)   # an_b broadcast
            nc.sync.dma_start(
                out=gB,
                in_=bass.AP(tensor=angb_d, offset=0, ap=[[0, 128], [1, A]]),
            )
            nc.sync.dma_start(
                out=bB,
                in_=bass.AP(tensor=angb_d, offset=A, ap=[[0, 128], [1, A]]),
            )
            agB = consts.tile([128, E, A], BF16)  # adapter_g (e-major) bcast
            nc.sync.dma_start(
                out=agB,
                in_=bass.AP(tensor=ag_d, offset=0, ap=[[0, 128], [A, E], [1, A]]),
            )
            ew_sb = consts.tile([128, SC, E], F32)
            nc.sync.dma_start(
                out=ew_sb, in_=ew_d[:].rearrange("(sc p) e -> p sc e", p=128)
            )
            bmix_sb = consts.tile([128, SC, A], BF16)
            nc.sync.dma_start(
                out=bmix_sb, in_=bmix_d[:].rearrange("(sc p) a -> p sc a", p=128)
            )

            # ---------- weight loads (split per chunk so consumers start early) ----
            ug_w = wpool.tile([128, KD, 2 * H], BF16)
            ug_src = ug_d[:].rearrange("(k p) h -> p k h", p=128)
            for k in range(KD):
                nc.sync.dma_start(out=ug_w[:, k, :], in_=ug_src[:, k, :])
            post_w = wpool.tile([128, HC, A], BF16)
            nc.sync.dma_start(
                out=post_w, in_=post_d[:].rearrange("(k p) a -> p k a", p=128)
            )
            adw = wpool.tile([128, E * A], BF16)
            nc.sync.dma_start(out=adw, in_=adw_d[:])
            wfin = wpool.tile([128, HC + 2, D], BF16)
            wfin_src = wfin_d[:].rearrange("(k p) d -> p k d", p=128)
            for k in range(HC + 2):
                nc.sync.dma_start(out=wfin[:, k, :], in_=wfin_src[:, k, :])

            # persistent activations
            AI_tok = acts.tile([128, SC, A], BF16)    # adapt_in, token-major
            AO_tok = acts.tile([128, SC, A], BF16)    # adapt_out, token-major
            AIT = acts.tile([128, T], BF16)           # adapt_in, feature-major
            hid = acts.tile([128, HC, T], BF16)       # hidden, feature-major
            AOTfull = acts.tile([128, GROUP, T], BF16)    # gathered AO feat-major
            adaptT = acts.tile([128, T], BF16)            # adapt, feature-major
            mixedT = acts.tile([128, T], BF16)            # mixed, feature-major
            mix_tok = acts.tile([128, SC, A], BF16)       # mixed, token-major

            def layernorm_to(ps, dst):
                """LN over free dim (A=128) of psum tile [128, A]; write dst bf16."""
                st = work.tile([128, 6], F32, tag="lnst")
                nc.vector.bn_stats(out=st, in_=ps)
                mv = work.tile([128, 2], F32, tag="lnmv")
                nc.vector.bn_aggr(out=mv, in_=st)
                sd = work.tile([128, 1], F32, tag="lnsd")
                nc.scalar.activation(
                    out=sd, in_=mv[:, 1:2], func=mybir.ActivationFunctionType.Sqrt,
                    bias=eps_t, scale=1.0,
                )
                r = work.tile([128, 1], F32, tag="lnr")
                nc.vector.reciprocal(out=r, in_=sd)
                z = work.tile([128, A], F32, tag="lnz")
                nc.vector.tensor_scalar(
                    out=z, in0=ps, scalar1=mv[:, 0:1], scalar2=r,
                    op0=mybir.AluOpType.subtract, op1=mybir.AluOpType.mult,
                )
                zg = work.tile([128, A], F32, tag="lnzg")
                nc.vector.tensor_tensor(out=zg, in0=z, in1=gB, op=mybir.AluOpType.mult)
                nc.vector.tensor_tensor(out=dst, in0=zg, in1=bB, op=mybir.AluOpType.add)

            # ---------- adapt_in = LN(x @ pre_w.T), then AllGather #1 ----------
            for sc in range(SC):
                ps = ps_sm.tile([128, A], F32, tag="sm")
                for k in range(KD):
                    nc.tensor.matmul(
                        ps, xT[:, k, sc * 128:(sc + 1) * 128], pre_w[:, k, :],
                        start=(k == 0), stop=(k == KD - 1),
                    )
                layernorm_to(ps, AI_tok[:, sc, :])
            nc.gpsimd.dma_start(
                out=ag1_in[:].rearrange("(sc p) a -> p sc a", p=128), in_=AI_tok
            )
            nc.gpsimd.collective_compute(
                "AllGather", mybir.AluOpType.bypass, replica_groups=RG,
                ins=[ag1_in[:]], outs=[ag1_out[:]],
            )
            AIfull = acts.tile([128, TC_FULL, A], BF16)   # gathered AI token-major
            nc.gpsimd.dma_start(
                out=AIfull, in_=ag1_out[:].rearrange("(k p) a -> p k a", p=128)
            )

            # transpose AI -> feature-major AIT
            for sc in range(SC):
                tp = ps_sm.tile([128, 128], BF16, tag="sm")
                nc.tensor.transpose(tp, AI_tok[:, sc, :], ident)
                nc.scalar.copy(out=AIT[:, sc * 128:(sc + 1) * 128], in_=tp)

            # ---------- hidden = silu(x@gate.T) * (x@up.T), feature-major ----------
            for hc in range(HC):
                up_ps = ps_big.tile([128, T], F32, tag="mm")
                gt_ps = ps_big.tile([128, T], F32, tag="mm")
                for k in range(KD):
                    nc.tensor.matmul(
                        up_ps, ug_w[:, k, hc * 128:(hc + 1) * 128], xT[:, k, :],
                        start=(k == 0), stop=(k == KD - 1),
                    )
                for k in range(KD):
                    nc.tensor.matmul(
                        gt_ps, ug_w[:, k, H + hc * 128:H + (hc + 1) * 128],
                        xT[:, k, :], start=(k == 0), stop=(k == KD - 1),
                    )
                sg = work2.tile([128, T], BF16, tag="sg")
                nc.scalar.activation(
                    out=sg, in_=gt_ps, func=mybir.ActivationFunctionType.Silu
                )
                nc.vector.tensor_tensor(
                    out=hid[:, hc, :], in0=sg, in1=up_ps, op=mybir.AluOpType.mult
                )

            # ---------- adapt_out = LN(hidden @ post_w.T), then AllGather #2 ----------
            for sc in range(SC):
                ps = ps_sm.tile([128, A], F32, tag="sm")
                for k in range(HC):
                    nc.tensor.matmul(
                        ps, hid[:, k, sc * 128:(sc + 1) * 128], post_w[:, k, :],
                        start=(k == 0), stop=(k == HC - 1),
                    )
                layernorm_to(ps, AO_tok[:, sc, :])
            AOT = acts.tile([128, T], BF16)
            for sc in range(SC):
                tp = ps_sm.tile([128, 128], BF16, tag="sm")
                nc.tensor.transpose(tp, AO_tok[:, sc, :], ident)
                nc.scalar.copy(out=AOT[:, sc * 128:(sc + 1) * 128], in_=tp)
            nc.gpsimd.dma_start(out=ag2_in[:], in_=AOT)
            nc.gpsimd.collective_compute(
                "AllGather", mybir.AluOpType.bypass, replica_groups=RG,
                ins=[ag2_in[:]], outs=[ag2_out[:]],
            )

            # ---------- expert path (local tokens only) ----------
            for sc in range(SC):
                hp0 = ps_big.tile([128, 512], F32, tag="mm")
                hp1 = ps_big.tile([128, 512], F32, tag="mm")
                sl = AIT[:, sc * 128:(sc + 1) * 128]
                nc.tensor.matmul(hp0, sl, adw[:, 0:512], start=True, stop=True)
                nc.tensor.matmul(hp1, sl, adw[:, 512:1024], start=True, stop=True)
                hps = [hp0, hp0, hp0, hp0, hp1, hp1, hp1, hp1]
                st8 = work.tile([128, E, 6], F32, tag="st8")
                for e in range(E):
                    nc.vector.bn_stats(
                        out=st8[:, e, :], in_=hps[e][:, (e % 4) * A:(e % 4 + 1) * A]
                    )
                mv8 = work.tile([128, E, 2], F32, tag="mv8")
                for e in range(E):
                    nc.vector.bn_aggr(out=mv8[:, e, :], in_=st8[:, e, :])
                sd8 = work.tile([128, E], F32, tag="sd8")
                nc.scalar.activation(
                    out=sd8, in_=mv8[:, :, 1], func=mybir.ActivationFunctionType.Sqrt,
                    bias=eps_t, scale=1.0,
                )
                r8 = work.tile([128, E], F32, tag="r8")
                nc.vector.reciprocal(out=r8, in_=sd8)
                rw8 = work.tile([128, E], F32, tag="rw8")
                nc.vector.tensor_tensor(
                    out=rw8, in0=r8, in1=ew_sb[:, sc, :], op=mybir.AluOpType.mult
                )
                nmrw = work.tile([128, E], F32, tag="nmrw")
                nc.vector.tensor_tensor(
                    out=nmrw, in0=mv8[:, :, 0], in1=rw8, op=mybir.AluOpType.mult
                )
                nc.vector.tensor_scalar(
                    out=nmrw, in0=nmrw, scalar1=-1.0, scalar2=None,
                    op0=mybir.AluOpType.mult,
                )
                # z~_e = h_e * (r*ew)_e - m*(r*ew)_e, written e-outer [s, e, c]
                zt = workbig.tile([128, E, A], BF16, tag="zt")
                for e in range(E):
                    nc.scalar.activation(
                        out=zt[:, e, :], in_=hps[e][:, (e % 4) * A:(e % 4 + 1) * A],
                        func=mybir.ActivationFunctionType.Identity,
                        scale=rw8[:, e:e + 1], bias=nmrw[:, e:e + 1],
                    )
                zg = workbig.tile([128, E, A], BF16, tag="ztg")
                nc.vector.tensor_tensor(
                    out=zg, in0=zt, in1=agB, op=mybir.AluOpType.mult
                )
                t1 = workbig.tile([128, 4, A], BF16, tag="sum1")
                nc.vector.tensor_tensor(
                    out=t1, in0=zg[:, 0:4, :], in1=zg[:, 4:8, :],
                    op=mybir.AluOpType.add,
                )
                t2 = work.tile([128, 2, A], BF16, tag="sum2")
                nc.vector.tensor_tensor(
                    out=t2, in0=t1[:, 0:2, :], in1=t1[:, 2:4, :],
                    op=mybir.AluOpType.add,
                )
                mx = work.tile([128, A], BF16, tag="mx")
                nc.vector.tensor_tensor(
                    out=mx, in0=t2[:, 0, :], in1=t2[:, 1, :], op=mybir.AluOpType.add
                )
                nc.vector.tensor_tensor(
                    out=mix_tok[:, sc, :], in0=mx, in1=bmix_sb[:, sc, :],
                    op=mybir.AluOpType.add,
                )
            for sc in range(SC):
                tp = ps_sm.tile([128, 128], BF16, tag="sm")
                nc.tensor.transpose(tp, mix_tok[:, sc, :], ident)
                nc.scalar.copy(out=mixedT[:, sc * 128:(sc + 1) * 128], in_=tp)

            # ---------- final output accumulation: first chunks (overlap AG) ----
            out_ps = {}

            def final_down(dc):
                op = ps_out.tile([128, T], F32, tag="fout")
                out_ps[dc] = op
                for k in range(HC):
                    nc.tensor.matmul(
                        op, wfin[:, k, dc * 128:(dc + 1) * 128], hid[:, k, :],
                        start=(k == 0), stop=False,
                    )

            def final_close(dc):
                op = out_ps[dc]
                nc.tensor.matmul(
                    op, wfin[:, HC, dc * 128:(dc + 1) * 128], adaptT,
                    start=False, stop=False,
                )
                nc.tensor.matmul(
                    op, wfin[:, HC + 1, dc * 128:(dc + 1) * 128], mixedT,
                    start=False, stop=True,
                )
                ob = evac.tile([128, T], F32, tag="ob")
                nc.scalar.copy(out=ob, in_=op)
                nc.sync.dma_start(out=out_d[dc * 128:(dc + 1) * 128, :], in_=ob)

            for dc in range(3):
                final_down(dc)

            # ---------- load gathered tensors ----------
            nc.gpsimd.dma_start(
                out=AOTfull, in_=ag2_out[:].rearrange("(c a) t -> a c t", a=128)
            )
            AOTf = AOTfull.rearrange("a c t -> a (c t)")

            # ---------- w = silu(clip(AI_loc @ AO_full.T)) ; adapt = w.T-chain ----
            ad_ps = ps_acc.tile([128, T], F32, tag="adps")
            for j in range(TC_FULL):
                w_ps = ps_big.tile([128, T], F32, tag="mm")
                nc.tensor.matmul(
                    w_ps, AOTf[:, j * 128:(j + 1) * 128], AIT, start=True, stop=True
                )
                wc = work2.tile([128, T], BF16, tag="wc")
                nc.vector.tensor_scalar(
                    out=wc, in0=w_ps, scalar1=-5.0, scalar2=5.0,
                    op0=mybir.AluOpType.max, op1=mybir.AluOpType.min,
                )
                wt = work2.tile([128, T], BF16, tag="wts")
                nc.scalar.activation(
                    out=wt, in_=wc, func=mybir.ActivationFunctionType.Silu
                )
                nc.tensor.matmul(
                    ad_ps, AIfull[:, j, :], wt,
                    start=(j == 0), stop=(j == TC_FULL - 1),
                )
            nc.scalar.copy(out=adaptT, in_=ad_ps)

            # ---------- finish output ----------
            for dc in range(3):
                final_close(dc)
            for dc in range(3, DC):
                final_down(dc)
                final_close(dc)

    nc.compile()
    return nc


def kernel(
    x, expert_weights, up_w, gate_w, down_w, pre_w, post_w, an_g, an_b,
    adapt_proj_w, adapter_w, adapter_g, adapter_b, expert_proj_w, output_proj_w,
):
    x = np.asarray(x, np.float32)
    expert_weights = np.asarray(expert_weights, np.float32)
    bf = ml_dtypes.bfloat16

    if "nc" not in _CACHE:
        _CACHE["nc"] = _build()
    nc = _CACHE["nc"]

    ug_wT = np.concatenate(
        [np.asarray(up_w, np.float32), np.asarray(gate_w, np.float32)], axis=0
    ).T.astype(bf)                                             # [D, 2H]
    pre_wT = np.asarray(pre_w, np.float32).T.astype(bf)        # [D, A]
    post_wT = np.asarray(post_w, np.float32).T.astype(bf)      # [H, A]
    adapter_wT = (
        np.asarray(adapter_w, np.float32).transpose(2, 0, 1).reshape(A, E * A)
    ).astype(bf)                                               # [A, E*A] (e-major)
    down_w = np.asarray(down_w, np.float32)
    w_da = 0.1 * (down_w @ np.asarray(adapt_proj_w, np.float32))       # [D, A]
    w_mo = np.asarray(output_proj_w, np.float32) @ np.asarray(
        expert_proj_w, np.float32
    )                                                                   # [D, A]
    wfin = np.concatenate([down_w.T, w_da.T, w_mo.T], axis=0).astype(bf)  # [2304, D]
    angb = np.stack(
        [np.asarray(an_g, np.float32), np.asarray(an_b, np.float32)], axis=0
    )                                                                   # [2, A]
    ag_row = np.asarray(adapter_g, np.float32).reshape(1, A * E).astype(bf)  # e-major
    bias_mix = (expert_weights @ np.asarray(adapter_b, np.float32)).astype(bf)

    xf = x.reshape(N, D)
    shared = {
        "ug_wT": ug_wT, "pre_wT": pre_wT, "post_wT": post_wT,
        "adapter_wT": adapter_wT, "wfin": wfin, "angb": angb, "ag_row": ag_row,
    }
    in_maps = []
    for c in range(NCORES):
        sl = slice(c * T, (c + 1) * T)
        in_maps.append(
            dict(
                shared,
                xT=np.ascontiguousarray(xf[sl].T).astype(bf),
                ew=np.ascontiguousarray(expert_weights[sl]),
                bias_mix=np.ascontiguousarray(bias_mix[sl]),
            )
        )

    res = run_bass_kernel_spmd(nc, in_maps, list(range(NCORES))).results
    out = np.empty((N, D), np.float32)
    for c in range(NCORES):
        out[c * T:(c + 1) * T] = res[c]["out"].T
    return out.reshape(B, S, D)


# revision 8
# speedup vs baseline: 1.1118x; 1.1118x over previous
"""Trainium2 Bass kernel for nn_ExpertGroup (moe_routing).

Contract: kernel(**inputs) takes FULL unsharded numpy inputs and returns the
FULL [2, 2048, 1024] fp32 output. Internally shards B*S=4096 tokens across
8 NeuronCores (512 tokens/core; cores 0-3 own batch 0, cores 4-7 batch 1),
replicates the small weights, and exchanges the per-batch adapter tensors
(adapt_in / adapt_out, [S,128] each) with two intra-group AllGathers.

All matmuls run in bf16 with fp32 PSUM accumulation. Activations are laid out
feature-major ([feature, token]) so every matmul contracts over partitions.

Host-side algebraic folds (exact, input-dependent, valid for any inputs):
  - up/gate weights concatenated into one [D, 2H] lhsT
  - shared_out + out = hidden @ down_w.T + adapt @ (0.1*down_w@adapt_proj_w).T
                      + mixed @ (output_proj_w@expert_proj_w).T
    -> single PSUM accumulation over 16+1+1 K-chunks of 128
  - sum_e ew[n,e]*adapter_b[e,:]  (LN bias term) = expert_weights @ adapter_b
"""

import sys

sys.path.insert(0, "/opt/trn_rl_repo")

import ml_dtypes
import numpy as np

import concourse.bass as bass
import concourse.mybir as mybir
import concourse.tile as tile
from concourse import bacc
from concourse.bass_utils import run_bass_kernel_spmd
from concourse.masks import make_identity

BF16 = mybir.dt.bfloat16
F32 = mybir.dt.float32

B, S, D, E = 2, 2048, 1024, 8
H = 2 * D          # 2048
A = H // 16        # 128
N = B * S          # 4096
NCORES = 8
T = N // NCORES    # 512 tokens per core
GROUP = 4          # cores per batch
SC = T // 128      # 4 s-chunks per core
DC = D // 128      # 8 d-chunks (output features)
HC = H // 128      # 16 h-chunks
KD = D // 128      # 8 k-chunks over D
TC_FULL = S // 128  # 16 token-chunks per batch
EPS = 1e-5

_CACHE = {}


def _build():
    nc = bacc.Bacc(None, num_devices=NCORES)

    # ---- kernel I/O (per-core) ----
    xT_d = nc.dram_tensor("xT", [D, T], BF16, kind="ExternalInput")
    ug_d = nc.dram_tensor("ug_wT", [D, 2 * H], BF16, kind="ExternalInput")
    pre_d = nc.dram_tensor("pre_wT", [D, A], BF16, kind="ExternalInput")
    post_d = nc.dram_tensor("post_wT", [H, A], BF16, kind="ExternalInput")
    adw_d = nc.dram_tensor("adapter_wT", [A, E * A], BF16, kind="ExternalInput")
    wfin_d = nc.dram_tensor("wfin", [H + 2 * A, D], BF16, kind="ExternalInput")
    ew_d = nc.dram_tensor("ew", [T, E], F32, kind="ExternalInput")
    angb_d = nc.dram_tensor("angb", [2, A], F32, kind="ExternalInput")
    ag_d = nc.dram_tensor("ag_row", [1, A * E], BF16, kind="ExternalInput")
    bmix_d = nc.dram_tensor("bias_mix", [T, A], BF16, kind="ExternalInput")
    out_d = nc.dram_tensor("out", [D, T], F32, kind="ExternalOutput")

    # ---- collective bounce buffers (internal DRAM) ----
    ag1_in = nc.dram_tensor("ag1_in", [T, A], BF16)
    ag1_out = nc.dram_tensor("ag1_out", [S, A], BF16)
    ag2_in = nc.dram_tensor("ag2_in", [A, T], BF16)
    ag2_out = nc.dram_tensor("ag2_out", [GROUP * A, T], BF16)
    RG = [[0, 1, 2, 3], [4, 5, 6, 7]]

    with tile.TileContext(nc) as tc:
        with (
            tc.tile_pool(name="consts", bufs=1) as consts,
            tc.tile_pool(name="wpool", bufs=1) as wpool,
            tc.tile_pool(name="acts", bufs=1) as acts,
            tc.tile_pool(name="work", bufs=4) as work,
            tc.tile_pool(name="work2", bufs=3) as work2,
            tc.tile_pool(name="workbig", bufs=2) as workbig,
            tc.tile_pool(name="evac", bufs=3) as evac,
            tc.tile_pool(name="ps_big", bufs=2, space="PSUM") as ps_big,
            tc.tile_pool(name="ps_acc", bufs=1, space="PSUM") as ps_acc,
            tc.tile_pool(name="ps_out", bufs=3, space="PSUM") as ps_out,
            tc.tile_pool(name="ps_sm", bufs=2, space="PSUM") as ps_sm,
        ):
            # ---------- priority loads: the pre-matmul chain needs these ----
            xT = wpool.tile([128, KD, T], BF16)
            pre_w = wpool.tile([128, KD, A], BF16)
            nc.sync.dma_start(out=xT, in_=xT_d[:].rearrange("(k p) t -> p k t", p=128))
            nc.sync.dma_start(
                out=pre_w, in_=pre_d[:].rearrange("(k p) a -> p k a", p=128)
            )

            # ---------- constants ----------
            ident = consts.tile([128, 128], BF16)
            make_identity(nc, ident)
            eps_t = consts.tile([128, 1], F32)
            nc.vector.memset(eps_t, EPS)
            gB = consts.tile([128, A], F32)   # an_g broadcast across partitions
            bB = consts.tile([128, A], F32)   # an_b broadcast
            nc.sync.dma_start(
                out=gB,
                in_=bass.AP(tensor=angb_d, offset=0, ap=[[0, 128], [1, A]]),
            )
            nc.sync.dma_start(
                out=bB,
                in_=bass.AP(tensor=angb_d, offset=A, ap=[[0, 128], [1, A]]),
            )
            agB = consts.tile([128, E, A], BF16)  # adapter_g (e-major) bcast
            nc.sync.dma_start(
                out=agB,
                in_=bass.AP(tensor=ag_d, offset=0, ap=[[0, 128], [A, E], [1, A]]),
            )
            ew_sb = consts.tile([128, SC, E], F32)
            nc.sync.dma_start(
                out=ew_sb, in_=ew_d[:].rearrange("(sc p) e -> p sc e", p=128)
            )
            bmix_sb = consts.tile([128, SC, A], BF16)
            nc.sync.dma_start(
                out=bmix_sb, in_=bmix_d[:].rearrange("(sc p) a -> p sc a", p=128)
            )

            # ---------- weight loads (split per chunk so consumers start early) ----
            ug_w = wpool.tile([128, KD, 2 * H], BF16)
            ug_src = ug_d[:].rearrange("(k p) h -> p k h", p=128)
            for k in range(KD):
                nc.sync.dma_start(out=ug_w[:, k, :], in_=ug_src[:, k, :])
            post_w = wpool.tile([128, HC, A], BF16)
            nc.sync.dma_start(
                out=post_w, in_=post_d[:].rearrange("(k p) a -> p k a", p=128)
            )
            adw = wpool.tile([128, E * A], BF16)
            nc.sync.dma_start(out=adw, in_=adw_d[:])
            wfin = wpool.tile([128, HC + 2, D], BF16)
            wfin_src = wfin_d[:].rearrange("(k p) d -> p k d", p=128)
            for k in range(HC + 2):
                nc.sync.dma_start(out=wfin[:, k, :], in_=wfin_src[:, k, :])

            # persistent activations
            AI_tok = acts.tile([128, SC, A], BF16)    # adapt_in, token-major
            AO_tok = acts.tile([128, SC, A], BF16)    # adapt_out, token-major
            AIT = acts.tile([128, T], BF16)           # adapt_in, feature-major
            hid = acts.tile([128, HC, T], BF16)       # hidden, feature-major
            AOTfull = acts.tile([128, GROUP, T], BF16)    # gathered AO feat-major
            adaptT = acts.tile([128, T], BF16)            # adapt, feature-major
            mixedT = acts.tile([128, T], BF16)            # mixed, feature-major
            mix_tok = acts.tile([128, SC, A], BF16)       # mixed, token-major

            def layernorm_to(ps, dst):
                """LN over free dim (A=128) of psum tile [128, A]; write dst bf16."""
                st = work.tile([128, 6], F32, tag="lnst")
                nc.vector.bn_stats(out=st, in_=ps)
                mv = work.tile([128, 2], F32, tag="lnmv")
                nc.vector.bn_aggr(out=mv, in_=st)
                sd = work.tile([128, 1], F32, tag="lnsd")
                nc.scalar.activation(
                    out=sd, in_=mv[:, 1:2], func=mybir.ActivationFunctionType.Sqrt,
                    bias=eps_t, scale=1.0,
                )
                r = work.tile([128, 1], F32, tag="lnr")
                nc.vector.reciprocal(out=r, in_=sd)
                z = work.tile([128, A], F32, tag="lnz")
                nc.vector.tensor_scalar(
                    out=z, in0=ps, scalar1=mv[:, 0:1], scalar2=r,
                    op0=mybir.AluOpType.subtract, op1=mybir.AluOpType.mult,
                )
                zg = work.tile([128, A], F32, tag="lnzg")
                nc.vector.tensor_tensor(out=zg, in0=z, in1=gB, op=mybir.AluOpType.mult)
                nc.vector.tensor_tensor(out=dst, in0=zg, in1=bB, op=mybir.AluOpType.add)

            # ---------- adapt_in = LN(x @ pre_w.T), then AllGather #1 ----------
            for sc in range(SC):
                ps = ps_sm.tile([128, A], F32, tag="sm")
                for k in range(KD):
                    nc.tensor.matmul(
                        ps, xT[:, k, sc * 128:(sc + 1) * 128], pre_w[:, k, :],
                        start=(k == 0), stop=(k == KD - 1),
                    )
                layernorm_to(ps, AI_tok[:, sc, :])
            nc.gpsimd.dma_start(
                out=ag1_in[:].rearrange("(sc p) a -> p sc a", p=128), in_=AI_tok
            )
            nc.gpsimd.collective_compute(
                "AllGather", mybir.AluOpType.bypass, replica_groups=RG,
                ins=[ag1_in[:]], outs=[ag1_out[:]],
            )
            AIfull = acts.tile([128, TC_FULL, A], BF16)   # gathered AI token-major
            nc.gpsimd.dma_start(
                out=AIfull, in_=ag1_out[:].rearrange("(k p) a -> p k a", p=128)
            )

            # transpose AI -> feature-major AIT
            for sc in range(SC):
                tp = ps_sm.tile([128, 128], BF16, tag="sm")
                nc.tensor.transpose(tp, AI_tok[:, sc, :], ident)
                nc.scalar.copy(out=AIT[:, sc * 128:(sc + 1) * 128], in_=tp)

            # ---------- hidden = silu(x@gate.T) * (x@up.T), feature-major ----------
            for hc in range(HC):
                up_ps = ps_big.tile([128, T], F32, tag="mm")
                gt_ps = ps_big.tile([128, T], F32, tag="mm")
                for k in range(KD):
                    nc.tensor.matmul(
                        up_ps, ug_w[:, k, hc * 128:(hc + 1) * 128], xT[:, k, :],
                        start=(k == 0), stop=(k == KD - 1),
                    )
                for k in range(KD):
                    nc.tensor.matmul(
                        gt_ps, ug_w[:, k, H + hc * 128:H + (hc + 1) * 128],
                        xT[:, k, :], start=(k == 0), stop=(k == KD - 1),
                    )
                sg = work2.tile([128, T], BF16, tag="sg")
                nc.scalar.activation(
                    out=sg, in_=gt_ps, func=mybir.ActivationFunctionType.Silu
                )
                nc.vector.tensor_tensor(
                    out=hid[:, hc, :], in0=sg, in1=up_ps, op=mybir.AluOpType.mult
                )

            # ---------- adapt_out = LN(hidden @ post_w.T), then AllGather #2 ----------
            for sc in range(SC):
                ps = ps_sm.tile([128, A], F32, tag="sm")
                for k in range(HC):
                    nc.tensor.matmul(
                        ps, hid[:, k, sc * 128:(sc + 1) * 128], post_w[:, k, :],
                        start=(k == 0), stop=(k == HC - 1),
                    )
                layernorm_to(ps, AO_tok[:, sc, :])
            AOT = acts.tile([128, T], BF16)
            for sc in range(SC):
                tp = ps_sm.tile([128, 128], BF16, tag="sm")
                nc.tensor.transpose(tp, AO_tok[:, sc, :], ident)
                nc.scalar.copy(out=AOT[:, sc * 128:(sc + 1) * 128], in_=tp)
            nc.gpsimd.dma_start(out=ag2_in[:], in_=AOT)
            nc.gpsimd.collective_compute(
                "AllGather", mybir.AluOpType.bypass, replica_groups=RG,
                ins=[ag2_in[:]], outs=[ag2_out[:]],
            )

            # ---------- expert path (local tokens only) ----------
            for sc in range(SC):
                hp0 = ps_big.tile([128, 512], F32, tag="mm")
                hp1 = ps_big.tile([128, 512], F32, tag="mm")
                sl = AIT[:, sc * 128:(sc + 1) * 128]
                nc.tensor.matmul(hp0, sl, adw[:, 0:512], start=True, stop=True)
                nc.tensor.matmul(hp1, sl, adw[:, 512:1024], start=True, stop=True)
                hps = [hp0, hp0, hp0, hp0, hp1, hp1, hp1, hp1]
                st8 = work.tile([128, E, 6], F32, tag="st8")
                for e in range(E):
                    nc.vector.bn_stats(
                        out=st8[:, e, :], in_=hps[e][:, (e % 4) * A:(e % 4 + 1) * A]
                    )
                mv8 = work.tile([128, E, 2], F32, tag="mv8")
                for e in range(E):
                    nc.vector.bn_aggr(out=mv8[:, e, :], in_=st8[:, e, :])
                sd8 = work.tile([128, E], F32, tag="sd8")
                nc.scalar.activation(
                    out=sd8, in_=mv8[:, :, 1], func=mybir.ActivationFunctionType.Sqrt,
                    bias=eps_t, scale=1.0,
                )
                r8 = work.tile([128, E], F32, tag="r8")
                nc.vector.reciprocal(out=r8, in_=sd8)
                rw8 = work.tile([128, E], F32, tag="rw8")
                nc.vector.tensor_tensor(
                    out=rw8, in0=r8, in1=ew_sb[:, sc, :], op=mybir.AluOpType.mult
                )
                nmrw = work.tile([128, E], F32, tag="nmrw")
                nc.vector.tensor_tensor(
                    out=nmrw, in0=mv8[:, :, 0], in1=rw8, op=mybir.AluOpType.mult
                )
                nc.vector.tensor_scalar(
                    out=nmrw, in0=nmrw, scalar1=-1.0, scalar2=None,
                    op0=mybir.AluOpType.mult,
                )
                # z~_e = h_e * (r*ew)_e - m*(r*ew)_e, written e-outer [s, e, c]
                zt = workbig.tile([128, E, A], BF16, tag="zt")
                for e in range(E):
                    nc.scalar.activation(
                        out=zt[:, e, :], in_=hps[e][:, (e % 4) * A:(e % 4 + 1) * A],
                        func=mybir.ActivationFunctionType.Identity,
                        scale=rw8[:, e:e + 1], bias=nmrw[:, e:e + 1],
                    )
                zg = workbig.tile([128, E, A], BF16, tag="ztg")
                nc.vector.tensor_tensor(
                    out=zg, in0=zt, in1=agB, op=mybir.AluOpType.mult
                )
                t1 = workbig.tile([128, 4, A], BF16, tag="sum1")
                nc.vector.tensor_tensor(
                    out=t1, in0=zg[:, 0:4, :], in1=zg[:, 4:8, :],
                    op=mybir.AluOpType.add,
                )
                t2 = work.tile([128, 2, A], BF16, tag="sum2")
                nc.vector.tensor_tensor(
                    out=t2, in0=t1[:, 0:2, :], in1=t1[:, 2:4, :],
                    op=mybir.AluOpType.add,
                )
                mx = work.tile([128, A], BF16, tag="mx")
                nc.vector.tensor_tensor(
                    out=mx, in0=t2[:, 0, :], in1=t2[:, 1, :], op=mybir.AluOpType.add
                )
                nc.vector.tensor_tensor(
                    out=mix_tok[:, sc, :], in0=mx, in1=bmix_sb[:, sc, :],
                    op=mybir.AluOpType.add,
                )
            for sc in range(SC):
                tp = ps_sm.tile([128, 128], BF16, tag="sm")
                nc.tensor.transpose(tp, mix_tok[:, sc, :], ident)
                nc.scalar.copy(out=mixedT[:, sc * 128:(sc + 1) * 128], in_=tp)

            # ---------- final output: down-part for ALL chunks into SBUF acc ----
            # (runs during the AllGather window; psum released per chunk)
            facc = acts.tile([128, DC, T], F32)

            def final_down(dc):
                op = ps_out.tile([128, T], F32, tag="fout")
                for k in range(HC):
                    nc.tensor.matmul(
                        op, wfin[:, k, dc * 128:(dc + 1) * 128], hid[:, k, :],
                        start=(k == 0), stop=(k == HC - 1),
                    )
                nc.scalar.copy(out=facc[:, dc, :], in_=op)

            def final_close(dc):
                op = ps_out.tile([128, T], F32, tag="fout")
                nc.tensor.matmul(
                    op, wfin[:, HC, dc * 128:(dc + 1) * 128], adaptT,
                    start=True, stop=False,
                )
                nc.tensor.matmul(
                    op, wfin[:, HC + 1, dc * 128:(dc + 1) * 128], mixedT,
                    start=False, stop=True,
                )
                ob = evac.tile([128, T], F32, tag="ob")
                nc.vector.tensor_tensor(
                    out=ob, in0=facc[:, dc, :], in1=op, op=mybir.AluOpType.add
                )
                nc.sync.dma_start(out=out_d[dc * 128:(dc + 1) * 128, :], in_=ob)

            for dc in range(DC):
                final_down(dc)

            # ---------- load gathered tensors ----------
            nc.gpsimd.dma_start(
                out=AOTfull, in_=ag2_out[:].rearrange("(c a) t -> a c t", a=128)
            )
            AOTf = AOTfull.rearrange("a c t -> a (c t)")

            # ---------- w = silu(clip(AI_loc @ AO_full.T)) ; adapt = w.T-chain ----
            ad_ps = ps_acc.tile([128, T], F32, tag="adps")
            for j in range(TC_FULL):
                w_ps = ps_big.tile([128, T], F32, tag="mm")
                nc.tensor.matmul(
                    w_ps, AOTf[:, j * 128:(j + 1) * 128], AIT, start=True, stop=True
                )
                wc = work2.tile([128, T], BF16, tag="wc")
                nc.vector.tensor_scalar(
                    out=wc, in0=w_ps, scalar1=-5.0, scalar2=5.0,
                    op0=mybir.AluOpType.max, op1=mybir.AluOpType.min,
                )
                wt = work2.tile([128, T], BF16, tag="wts")
                nc.scalar.activation(
                    out=wt, in_=wc, func=mybir.ActivationFunctionType.Silu
                )
                nc.tensor.matmul(
                    ad_ps, AIfull[:, j, :], wt,
                    start=(j == 0), stop=(j == TC_FULL - 1),
                )
            nc.scalar.copy(out=adaptT, in_=ad_ps)

            # ---------- finish output ----------
            for dc in range(DC):
                final_close(dc)

    nc.compile()
    return nc


def kernel(
    x, expert_weights, up_w, gate_w, down_w, pre_w, post_w, an_g, an_b,
    adapt_proj_w, adapter_w, adapter_g, adapter_b, expert_proj_w, output_proj_w,
):
    x = np.asarray(x, np.float32)
    expert_weights = np.asarray(expert_weights, np.float32)
    bf = ml_dtypes.bfloat16

    if "nc" not in _CACHE:
        _CACHE["nc"] = _build()
    nc = _CACHE["nc"]

    ug_wT = np.concatenate(
        [np.asarray(up_w, np.float32), np.asarray(gate_w, np.float32)], axis=0
    ).T.astype(bf)                                             # [D, 2H]
    pre_wT = np.asarray(pre_w, np.float32).T.astype(bf)        # [D, A]
    post_wT = np.asarray(post_w, np.float32).T.astype(bf)      # [H, A]
    adapter_wT = (
        np.asarray(adapter_w, np.float32).transpose(2, 0, 1).reshape(A, E * A)
    ).astype(bf)                                               # [A, E*A] (e-major)
    down_w = np.asarray(down_w, np.float32)
    w_da = 0.1 * (down_w @ np.asarray(adapt_proj_w, np.float32))       # [D, A]
    w_mo = np.asarray(output_proj_w, np.float32) @ np.asarray(
        expert_proj_w, np.float32
    )                                                                   # [D, A]
    wfin = np.concatenate([down_w.T, w_da.T, w_mo.T], axis=0).astype(bf)  # [2304, D]
    angb = np.stack(
        [np.asarray(an_g, np.float32), np.asarray(an_b, np.float32)], axis=0
    )                                                                   # [2, A]
    ag_row = np.asarray(adapter_g, np.float32).reshape(1, A * E).astype(bf)  # e-major
    bias_mix = (expert_weights @ np.asarray(adapter_b, np.float32)).astype(bf)

    xf = x.reshape(N, D)
    shared = {
        "ug_wT": ug_wT, "pre_wT": pre_wT, "post_wT": post_wT,
        "adapter_wT": adapter_wT, "wfin": wfin, "angb": angb, "ag_row": ag_row,
    }
    in_maps = []
    for c in range(NCORES):
        sl = slice(c * T, (c + 1) * T)
        in_maps.append(
            dict(
                shared,
                xT=np.ascontiguousarray(xf[sl].T).astype(bf),
                ew=np.ascontiguousarray(expert_weights[sl]),
                bias_mix=np.ascontiguousarray(bias_mix[sl]),
            )
        )

    res = run_bass_kernel_spmd(nc, in_maps, list(range(NCORES))).results
    out = np.empty((N, D), np.float32)
    for c in range(NCORES):
        out[c * T:(c + 1) * T] = res[c]["out"].T
    return out.reshape(B, S, D)
